# revision 4
# baseline (speedup 1.0000x reference)
"""Trainium2 Bass kernel for nn_BModel (BinaryLinear: out = x @ sign(W).T / sqrt(in_dim)).

Strategy (data-parallel over 8 NeuronCores):
  - x [4096, 32768] f32 is sharded along batch: 512 rows per core.
  - W [100, 32768] f32 is host-transposed (pure layout marshalling) to
    wt = W.T [32768, 100] and replicated to every core; sign() is computed
    on-device.

Per-core kernel:
  - k is decomposed as k = rh*(128*128) + p*128 + j  (rh in [0,2), p = SBUF
    partition, j in [0,128)).  With this decomposition the transposed-x
    operand the TensorEngine needs (contraction on partitions) is produced
    purely by a strided DMA access pattern whose HBM-side runs are 512 B
    contiguous -- no on-chip transpose at all.
  - x tiles are loaded with a casting SWDGE DMA (f32 -> fp16).  sign(W) is
    exactly representable in fp16 and PSUM accumulates in f32, so the only
    error is the fp16 rounding of x (~3e-4 relative).
  - sign(wt) is computed on ScalarE with the Sign activation, pre-scaled by
    2^64 so LUT behaviour near zero cannot matter; sign(0)=0 matches
    jnp.sign exactly.
  - Matmuls: psum[c, b] += sum_p w_sT[p, c] * xT[p, b], accumulating over
    all 256 (rh, j) contraction chunks; evacuated with a fused 1/sqrt(K)
    scale on ScalarE; output is written transposed [100, B] and the host
    transposes it back.
"""

import math

import numpy as np

N_CORES = 8
BATCH = 4096
K = 32768
C = 100
P = 128  # SBUF partitions
J = 128  # contiguous k elements per partition chunk (512 B f32 runs)
RH = K // (P * J)  # 2
B_PER_CORE = BATCH // N_CORES  # 512

_NC_CACHE = {}


def _build_nc(b_per_core=B_PER_CORE, bn=128, x_bufs=3):
    """Build + compile the per-core Bass program (identical on all cores)."""
    from contextlib import ExitStack

    import concourse.bass as bass
    import concourse.tile as tile
    from concourse import bacc, mybir

    f32 = mybir.dt.float32
    f16 = mybir.dt.float16

    bb_count = b_per_core // bn

    nc = bacc.Bacc(
        "TRN2",
        target_bir_lowering=False,
        debug=False,
        num_devices=N_CORES,
    )

    x = nc.dram_tensor("x", [b_per_core, K], f32, kind="ExternalInput").ap()
    wt = nc.dram_tensor("wt", [K, C], f32, kind="ExternalInput").ap()
    out_t = nc.dram_tensor("out_t", [C, b_per_core], f32, kind="ExternalOutput").ap()

    # k = rh*(P*J) + p*J + j
    x_view = x.rearrange("(bb b) (rh p j) -> bb rh p b j", bb=bb_count, rh=RH, p=P, j=J)
    wt_view = wt.rearrange("(rh p j) c -> p rh j c", rh=RH, p=P, j=J)

    scale = 1.0 / math.sqrt(K)

    with tile.TileContext(nc) as tc, ExitStack() as ctx:
        wpool = ctx.enter_context(tc.tile_pool(name="w", bufs=1))
        wtmp_pool = ctx.enter_context(tc.tile_pool(name="wtmp", bufs=2))
        xpool = ctx.enter_context(tc.tile_pool(name="x", bufs=x_bufs))
        psum_pool = ctx.enter_context(
            tc.tile_pool(name="psum", bufs=2, space="PSUM")
        )
        opool = ctx.enter_context(tc.tile_pool(name="o", bufs=2))

        # --- W prep: w_sT[p, rh, j, c] = sign(wt[rh*P*J + p*J + j, c]) in fp16
        w_sT = wpool.tile([P, RH, J, C], f16)
        JC = 16  # j-chunk for W prep
        for rh in range(RH):
            for j0 in range(0, J, JC):
                wtmp = wtmp_pool.tile([P, JC, C], f32)
                nc.sync.dma_start(wtmp[:], wt_view[:, rh, j0 : j0 + JC, :])
                # scale by 2^64 so the Sign LUT is only evaluated far from 0
                # (or at exactly 0); sign(0) = 0 matching jnp.sign.
                nc.scalar.activation(
                    w_sT[:, rh, j0 : j0 + JC, :],
                    wtmp[:],
                    mybir.ActivationFunctionType.Sign,
                    scale=float(2.0**64),
                )

        # --- main loop
        for bb in range(bb_count):
            psum = psum_pool.tile([C, bn], f32)
            for rh in range(RH):
                xt = xpool.tile([P, bn, J], f16)
                # casting DMA (SWDGE): f32 HBM -> fp16 SBUF, transposed layout.
                # Split over b to stay under the 16384-descriptor DMA cap
                # (one descriptor per (p, b) contiguous 512 B run).
                bs = max(1, (P * bn) // 8192)
                for s in range(bs):
                    b0, b1 = s * bn // bs, (s + 1) * bn // bs
                    nc.gpsimd.dma_start(
                        xt[:, b0:b1, :], x_view[bb, rh, :, b0:b1, :]
                    )
                for j in range(J):
                    nc.tensor.matmul(
                        psum[:, :],
                        w_sT[:, rh, j, :],
                        xt[:, :, j],
                        start=(rh == 0 and j == 0),
                        stop=(rh == RH - 1 and j == J - 1),
                    )
            ot = opool.tile([C, bn], f32)
            nc.scalar.activation(
                ot[:], psum[:, :], mybir.ActivationFunctionType.Copy, scale=scale
            )
            nc.sync.dma_start(out_t[:, bb * bn : (bb + 1) * bn], ot[:])

    nc.compile()
    return nc


def _get_nc(b_per_core=B_PER_CORE, bn=128, x_bufs=3):
    key = (b_per_core, bn, x_bufs)
    if key not in _NC_CACHE:
        _NC_CACHE[key] = _build_nc(*key)
    return _NC_CACHE[key]


def kernel(x, W, **run_kwargs):
    from concourse import bass_utils

    x = np.ascontiguousarray(np.asarray(x, dtype=np.float32))
    W = np.asarray(W, dtype=np.float32)
    wt = np.ascontiguousarray(W.T)  # [K, C], pure layout change

    nc = _get_nc()
    in_maps = [
        {"x": x[c * B_PER_CORE : (c + 1) * B_PER_CORE], "wt": wt}
        for c in range(N_CORES)
    ]
    res = bass_utils.run_bass_kernel_spmd(
        nc, in_maps, core_ids=list(range(N_CORES)), **run_kwargs
    )
    out = np.concatenate([r["out_t"].T for r in res.results], axis=0)
    if run_kwargs:
        return out, res
    return out
